# revision 3
# baseline (speedup 1.0000x reference)
"""Trainium2 Bass kernel: 3x3 VALID conv2d, stride 1.

Full input [32, 64, 112, 112] f32 + weights [128, 64, 3, 3] f32
-> output [32, 128, 110, 110] f32.

Data-parallel across 8 NeuronCores: 4 images per core.

Per-core formulation: conv as PE matmuls, out = lhsT.T @ rhs with
K (contraction, partitions) = 128 = 64 channels x 2 shifted copies,
M (out partitions) = 128 output channels,
N (moving free dim) = up to 4 input-width rows = 448 (<= 512, one PSUM
bank). The 2 rightmost columns of each 112-wide row are conv garbage;
the PSUM->SBUF copy compacts to the valid 110 columns.

FIVE matmuls per chunk cover all 9 taps (vs 6 for the naive row-pair
scheme) using two per-image SBUF planes:
  T tile: partitions 0-63 = image rows 0..110 (A), 64-127 = rows
          1..111 (B = A shifted one row).  Matmuls m=0..2 at column
          offset kx apply tap pairs (0,kx)+(1,kx).
  U tile: partitions 0-63 = rows 2..111 (C), 64-127 = rows 2..111
          shifted one column (C+1col).  m=3 applies (2,0)+(2,1) in one
          full-K matmul; m=4 applies (2,2) on the top half only
          (bottom-half weights zero) -- the single unavoidable
          half-waste (9 taps is odd).
U is built on-device from T's bottom half by two contiguous SBUF->SBUF
DMAs per band (flat shifts of +112 and +113 elements), so HBM input
traffic stays at the single-copy 12.8 MB/core.

Moving-N per tap is trimmed (n, n-1, n-2, n-1, n-2) so no rhs read
spills past input row y0+3: only garbage output columns lose taps.

Inputs are cast to fp16 on the host (fp32 PE is 4x slower; fp32 PSUM
accumulation keeps rel err ~3e-4).  Output is stored fp16 and cast
back to fp32 on the host, halving output HBM traffic.

A short burst of dummy matmuls on a memset tile runs during the DMA
startup window to flip the PE HAM clock gate to 2.4 GHz before real
work arrives.

Schedule: chunks are processed in groups of 8 across the 8 PSUM banks,
weight-plane-major (m outer), so consecutive matmuls hit different
banks (drain overlaps fill) and reuse the same stationary weights.
Engine split: gpsimd issues x loads, scalar builds U, vector does the
PSUM compaction copies (scalar helps on the last groups), sync does
the per-2-chunk output DMAs.
"""

import numpy as np

B_FULL = 32
N_CORES = 8
B_CORE = B_FULL // N_CORES  # 4 images per core
C_IN = 64
C_OUT = 128
H = W = 112
OH = OW = 110
TLEN = 111 * W  # 12432: T plane rows 0..110 (A) / 1..111 (B)
ULEN = 110 * W  # 12320: U plane rows 2..111

_NC = None


def _img_chunks():
    # per image: 27 chunks of 4 output rows + 1 of 2 rows = 110
    rows_list = [4] * 27 + [2]
    out = []
    y0 = 0
    for r in rows_list:
        out.append((y0, r))
        y0 += r
    assert y0 == OH
    return out


def _build():
    from contextlib import ExitStack

    import concourse.tile as tile
    from concourse import bacc, mybir

    nc = bacc.Bacc("TRN2", target_bir_lowering=False, debug=False)
    x = nc.dram_tensor(
        "x", [B_CORE, 128, TLEN], mybir.dt.float16, kind="ExternalInput"
    )
    w = nc.dram_tensor("w", [128, 5, 128], mybir.dt.float16, kind="ExternalInput")
    y = nc.dram_tensor(
        "y", [B_CORE, C_OUT, OH * OW], mybir.dt.float16, kind="ExternalOutput"
    )

    chunks = [(b, y0, r) for b in range(B_CORE) for (y0, r) in _img_chunks()]
    assert len(chunks) % 8 == 0
    n_groups = len(chunks) // 8

    with tile.TileContext(nc) as tc, ExitStack() as ctx:
        tpool = ctx.enter_context(tc.tile_pool(name="tp", bufs=B_CORE))
        upool = ctx.enter_context(tc.tile_pool(name="up", bufs=B_CORE))
        wpool = ctx.enter_context(tc.tile_pool(name="wp", bufs=1))
        spool = ctx.enter_context(tc.tile_pool(name="sp", bufs=1))
        opool = ctx.enter_context(tc.tile_pool(name="op", bufs=6))
        ppool = ctx.enter_context(tc.tile_pool(name="pp", bufs=8, space="PSUM"))

        wt = wpool.tile([128, 5 * 128], mybir.dt.float16)
        nc.sync.dma_start(wt[:], w.ap().rearrange("p a b -> p (a b)"))

        # PE warmup: HAM clock gate flips to 2.4 GHz after ~3.4us of
        # sustained activity; burn that in while the first x bands load.
        wu = spool.tile([128, 64], mybir.dt.float16)
        nc.gpsimd.memset(wu[:], 0)
        wu_p = ppool.tile([128, 448], mybir.dt.float32, name="wu_p", tag="pt")
        for _ in range(24):
            nc.tensor.matmul(
                wu_p[0:64, 0:64], wu[:], wu[:],
                start=True, stop=True, skip_group_check=True,
            )

        xa = x.ap()
        ya = y.ap()

        # Banded loads so the first chunks start early.  U band k reads
        # exactly T band k (U edges = T edges - 1).
        TB = [0, 9, 34, 61, 89, 111]
        UB = [0, 8, 33, 60, 88, 110]
        t_tiles, u_tiles = [], []
        for b in range(B_CORE):
            t = tpool.tile([128, TLEN], mybir.dt.float16, tag="t")
            u = upool.tile([128, ULEN], mybir.dt.float16, tag="u")
            for lo, hi in zip(TB, TB[1:]):
                nc.gpsimd.dma_start(
                    t[:, lo * W : hi * W], xa[b][:, lo * W : hi * W]
                )
            for lo, hi in zip(UB, UB[1:]):
                # U lo half: rows 2..111 = T bottom half shifted +1 row
                nc.scalar.dma_start(
                    u[0:64, lo * W : hi * W],
                    t[64:128, (lo + 1) * W : (hi + 1) * W],
                )
                # U hi half: same shifted one more column (+113 flat);
                # last band stops 1 elem short of TLEN (elem never read)
                e = min(hi * W, TLEN - 113)
                nc.scalar.dma_start(
                    u[64:128, lo * W : e],
                    t[64:128, lo * W + 113 : e + 113],
                )
            t_tiles.append(t)
            u_tiles.append(u)

        for g in range(n_groups):
            gchunks = chunks[g * 8 : (g + 1) * 8]
            pts = [
                ppool.tile([128, 448], mybir.dt.float32, name="pt", tag="pt")
                for _ in range(8)
            ]
            for m in range(5):
                for j, (b, y0, rows) in enumerate(gchunks):
                    n = rows * W
                    if m < 3:
                        nmv = n - m
                        rhs = t_tiles[b][:, y0 * W + m : y0 * W + m + nmv]
                    elif m == 3:
                        nmv = n - 1
                        rhs = u_tiles[b][:, y0 * W : y0 * W + nmv]
                    else:
                        nmv = n - 2
                        rhs = u_tiles[b][:, y0 * W + 1 : y0 * W + 1 + nmv]
                    nc.tensor.matmul(
                        pts[j][:, 0:nmv],
                        wt[:, m * 128 : (m + 1) * 128],
                        rhs,
                        start=(m == 0),
                        stop=(m == 4),
                        skip_group_check=True,
                    )
            # compact + store per 2 chunks: copies start draining PSUM as
            # soon as each pair of banks stops; ~0.22MB DMAs keep HWDGE
            # efficient without delaying the tail
            for h in range(4):
                pair = gchunks[2 * h : 2 * h + 2]
                total_rows = sum(r for _, _, r in pair)
                ot = opool.tile([128, 8 * OW], mybir.dt.float16, tag="ot")
                off = 0
                for jj, (b, y0, rows) in enumerate(pair):
                    j = 2 * h + jj
                    psrc = pts[j][:].rearrange("p (r c) -> p r c", c=W)[
                        :, 0:rows, 0:OW
                    ]
                    odst = ot[:, off : off + rows * OW].rearrange(
                        "p (r c) -> p r c", c=OW
                    )
                    # vector carries the steady state (scalar is busy
                    # issuing U builds); scalar joins for the tail
                    if g >= n_groups - 2 and jj == 1:
                        nc.scalar.copy(odst, psrc)
                    else:
                        nc.vector.tensor_copy(odst, psrc)
                    off += rows * OW
                b0, y00, _ = pair[0]
                assert all(b == b0 for b, _, _ in pair)
                nc.sync.dma_start(
                    ya[b0][:, y00 * OW : y00 * OW + total_rows * OW],
                    ot[:, 0 : total_rows * OW],
                )

    nc.compile()
    return nc


def _get_nc():
    global _NC
    if _NC is None:
        _NC = _build()
    return _NC


def _prep_weights(weights: np.ndarray) -> np.ndarray:
    # w5[c,     m, co] = w[co, c, 0, m] (m<3) ; w5[c,    3, co] = w[co, c, 2, 0]
    # w5[64+c,  m, co] = w[co, c, 1, m] (m<3) ; w5[64+c, 3, co] = w[co, c, 2, 1]
    # w5[c, 4, co] = 0                        ; w5[64+c, 4, co] = w[co, c, 2, 2]
    w = np.asarray(weights, dtype=np.float32)
    wt = w.transpose(1, 2, 3, 0)  # [ci, ky, kx, co]
    w5 = np.zeros((128, 5, 128), np.float32)
    w5[0:64, 0:3, :] = wt[:, 0, :, :]
    w5[64:128, 0:3, :] = wt[:, 1, :, :]
    w5[0:64, 3, :] = wt[:, 2, 0, :]
    w5[64:128, 3, :] = wt[:, 2, 1, :]
    w5[64:128, 4, :] = wt[:, 2, 2, :]
    return w5.astype(np.float16)


def kernel(input_image: np.ndarray, weights: np.ndarray, _trace: bool = False):
    from concourse.bass_utils import run_bass_kernel_spmd

    nc = _get_nc()
    x16 = np.asarray(input_image).astype(np.float16)
    r = x16.reshape(B_FULL, C_IN, H * W)
    xd = np.empty((B_FULL, 128, TLEN), np.float16)
    xd[:, 0:64] = r[:, :, :TLEN]  # A: rows 0..110
    xd[:, 64:128] = r[:, :, W : W + TLEN]  # B: rows 1..111
    w5 = _prep_weights(weights)
    in_maps = [
        {"x": xd[B_CORE * i : B_CORE * (i + 1)], "w": w5} for i in range(N_CORES)
    ]
    res = run_bass_kernel_spmd(
        nc, in_maps, core_ids=list(range(N_CORES)), trace=_trace
    )
    out = np.concatenate([res.results[i]["y"] for i in range(N_CORES)], axis=0)
    out = out.reshape(B_FULL, C_OUT, OH, OW).astype(np.float32)
    if _trace:
        return out, res
    return out


# revision 12
# speedup vs baseline: 1.0411x; 1.0411x over previous
"""Trainium2 Bass kernel: 3x3 VALID conv2d, stride 1.

Full input [32, 64, 112, 112] f32 + weights [128, 64, 3, 3] f32
-> output [32, 128, 110, 110] f32.

Data-parallel across 8 NeuronCores: 4 images per core.

Per-core formulation: conv as PE matmuls, out = lhsT.T @ rhs with
K (contraction, partitions) = 128 = 64 channels x 2 shifted copies,
M (out partitions) = 128 output channels,
N (moving free dim) = up to 4 input-width rows = 448 (<= 512, one PSUM
bank). The 2 rightmost columns of each 112-wide row are conv garbage;
the PSUM->SBUF copy compacts to the valid 110 columns.

FIVE matmuls per chunk cover all 9 taps (vs 6 for the naive row-pair
scheme) using two per-image SBUF planes:
  T tile: partitions 0-63 = image rows 0..110 (A), 64-127 = rows
          1..111 (B = A shifted one row).  Matmuls m=0..2 at column
          offset kx apply tap pairs (0,kx)+(1,kx).
  U tile: partitions 0-63 = rows 2..111 (C), 64-127 = rows 2..111
          shifted one column (C+1col).  m=3 applies (2,0)+(2,1) in one
          full-K matmul; m=4 applies (2,2) on the top half only
          (bottom-half weights zero) -- the single unavoidable
          half-waste (9 taps is odd).
U is built on-device by two SBUF->SBUF DMAs per band: the lo half
sources from T partitions 0-63 (A extended to all 112 rows, flat shift
+224) and the hi half from T partitions 64-127 (flat shift +113), so
the two copies use disjoint SBUF port halves and run in parallel at
full rate.  HBM input traffic stays at the single-copy ~12.9 MB/core.
(Streaming both planes from HBM was tried and saturates the ~358 GB/s
HBM interface: both input queues drop to ~140 GB/s and the PE starves.
Sourcing both halves from T[64:128] was also tried: the two copies then
fight over 8 ports and crawl at ~116 GB/s combined.)

Moving-N per tap is trimmed (n, n-1, n-2, n-1, n-2) so no rhs read
spills past input row y0+3: only garbage output columns lose taps.

Inputs are cast to fp16 on the host (fp32 PE is 4x slower; fp32 PSUM
accumulation keeps rel err ~3e-4).  Output is stored fp16 and cast
back to fp32 on the host, halving output HBM traffic.

A short burst of dummy matmuls on a memset tile runs during the DMA
startup window to flip the PE HAM clock gate to 2.4 GHz before real
work arrives.

Schedule: chunks are processed in groups of 8 across the 8 PSUM banks,
weight-plane-major (m outer), so consecutive matmuls hit different
banks (drain overlaps fill) and reuse the same stationary weights.
Engine split: gpsimd issues x loads, scalar builds U, vector does the
PSUM compaction copies (scalar helps on the last groups), sync does
the per-2-chunk output DMAs.
"""

import numpy as np

B_FULL = 32
N_CORES = 8
B_CORE = B_FULL // N_CORES  # 4 images per core
C_IN = 64
C_OUT = 128
H = W = 112
OH = OW = 110
TLEN = 112 * W  # 12544: T plane rows 0..111 (A) / 1..111 + zero pad (B)
ULEN = 110 * W  # 12320: U plane rows 2..111

_NC = None


def _img_chunks():
    # per image: 27 chunks of 4 output rows + 1 of 2 rows = 110
    rows_list = [4] * 27 + [2]
    out = []
    y0 = 0
    for r in rows_list:
        out.append((y0, r))
        y0 += r
    assert y0 == OH
    return out


def _build():
    from contextlib import ExitStack

    import concourse.tile as tile
    from concourse import bacc, mybir

    nc = bacc.Bacc("TRN2", target_bir_lowering=False, debug=False)
    x = nc.dram_tensor(
        "x", [B_CORE, 128, TLEN], mybir.dt.float16, kind="ExternalInput"
    )
    w = nc.dram_tensor("w", [128, 5, 128], mybir.dt.float16, kind="ExternalInput")
    y = nc.dram_tensor(
        "y", [B_CORE, C_OUT, OH * OW], mybir.dt.float16, kind="ExternalOutput"
    )

    chunks = [(b, y0, r) for b in range(B_CORE) for (y0, r) in _img_chunks()]
    assert len(chunks) % 8 == 0
    n_groups = len(chunks) // 8

    with tile.TileContext(nc) as tc, ExitStack() as ctx:
        tpool = ctx.enter_context(tc.tile_pool(name="tp", bufs=B_CORE))
        upool = ctx.enter_context(tc.tile_pool(name="up", bufs=B_CORE))
        wpool = ctx.enter_context(tc.tile_pool(name="wp", bufs=1))
        spool = ctx.enter_context(tc.tile_pool(name="sp", bufs=1))
        opool = ctx.enter_context(tc.tile_pool(name="op", bufs=6))
        ppool = ctx.enter_context(tc.tile_pool(name="pp", bufs=8, space="PSUM"))

        wt = wpool.tile([128, 5 * 128], mybir.dt.float16)
        nc.sync.dma_start(wt[:], w.ap().rearrange("p a b -> p (a b)"))

        # PE warmup: HAM clock gate flips to 2.4 GHz after ~3.4us of
        # sustained activity; burn that in while the first x bands load.
        wu = spool.tile([128, 128], mybir.dt.float16)
        nc.gpsimd.memset(wu[:], 0)
        wu_p = ppool.tile([128, 448], mybir.dt.float32, name="wu_p", tag="pt")
        for _ in range(14):
            nc.tensor.matmul(
                wu_p[0:64, 0:128], wu[:, 0:64], wu[:],
                start=True, stop=True, skip_group_check=True,
            )

        xa = x.ap()
        ya = y.ap()

        # Banded loads so the first chunks start early.  Image 0's first
        # bands ride the sync queue (earliest to start); the bulk of T
        # streams on gpsimd; U is built per-band on the scalar ring as
        # soon as the matching T band lands (U band k reads only T band k).
        t_tiles, u_tiles = [], []
        for b in range(B_CORE):
            t = tpool.tile([128, TLEN], mybir.dt.float16, tag="t")
            u = upool.tile([128, ULEN], mybir.dt.float16, tag="u")
            if b == 0:
                TB = [0, 6, 16, 34, 61, 89, 112]
                UB = [0, 4, 14, 32, 59, 87, 110]
            else:
                TB = [0, 34, 61, 89, 112]
                UB = [0, 32, 59, 87, 110]
            for k, (lo, hi) in enumerate(zip(TB, TB[1:])):
                eng = nc.sync if (b == 0 and k < 2) else nc.gpsimd
                eng.dma_start(t[:, lo * W : hi * W], xa[b][:, lo * W : hi * W])
            for lo, hi in zip(UB, UB[1:]):
                nc.scalar.dma_start(
                    u[0:64, lo * W : hi * W],
                    t[0:64, (lo + 2) * W : (hi + 2) * W],
                )
                nc.scalar.dma_start(
                    u[64:128, lo * W : hi * W],
                    t[64:128, lo * W + 113 : hi * W + 113],
                )
            t_tiles.append(t)
            u_tiles.append(u)

        for g in range(n_groups):
            gchunks = chunks[g * 8 : (g + 1) * 8]
            pts = [
                ppool.tile([128, 448], mybir.dt.float32, name="pt", tag="pt")
                for _ in range(8)
            ]
            for m in range(5):
                for j, (b, y0, rows) in enumerate(gchunks):
                    n = rows * W
                    if m < 3:
                        nmv = n - m
                        rhs = t_tiles[b][:, y0 * W + m : y0 * W + m + nmv]
                    elif m == 3:
                        nmv = n - 1
                        rhs = u_tiles[b][:, y0 * W : y0 * W + nmv]
                    else:
                        nmv = n - 2
                        rhs = u_tiles[b][:, y0 * W + 1 : y0 * W + 1 + nmv]
                    nc.tensor.matmul(
                        pts[j][:, 0:nmv],
                        wt[:, m * 128 : (m + 1) * 128],
                        rhs,
                        start=(m == 0),
                        stop=(m == 4),
                        skip_group_check=True,
                    )
            # compact + store per 2 chunks: copies start draining PSUM as
            # soon as each pair of banks stops; ~0.22MB DMAs keep HWDGE
            # efficient without delaying the tail
            for h in range(4):
                pair = gchunks[2 * h : 2 * h + 2]
                total_rows = sum(r for _, _, r in pair)
                ot = opool.tile([128, 8 * OW], mybir.dt.float16, tag="ot")
                off = 0
                for jj, (b, y0, rows) in enumerate(pair):
                    j = 2 * h + jj
                    psrc = pts[j][:].rearrange("p (r c) -> p r c", c=W)[
                        :, 0:rows, 0:OW
                    ]
                    odst = ot[:, off : off + rows * OW].rearrange(
                        "p (r c) -> p r c", c=OW
                    )
                    # vector carries the steady state (scalar is busy
                    # issuing U builds); scalar joins for the tail
                    if g >= n_groups - 2 and jj == 1:
                        nc.scalar.copy(odst, psrc)
                    else:
                        nc.vector.tensor_copy(odst, psrc)
                    off += rows * OW
                b0, y00, _ = pair[0]
                assert all(b == b0 for b, _, _ in pair)
                nc.sync.dma_start(
                    ya[b0][:, y00 * OW : y00 * OW + total_rows * OW],
                    ot[:, 0 : total_rows * OW],
                )

    nc.compile()
    return nc


def _get_nc():
    global _NC
    if _NC is None:
        _NC = _build()
    return _NC


def _prep_weights(weights: np.ndarray) -> np.ndarray:
    # w5[c,     m, co] = w[co, c, 0, m] (m<3) ; w5[c,    3, co] = w[co, c, 2, 0]
    # w5[64+c,  m, co] = w[co, c, 1, m] (m<3) ; w5[64+c, 3, co] = w[co, c, 2, 1]
    # w5[c, 4, co] = 0                        ; w5[64+c, 4, co] = w[co, c, 2, 2]
    w = np.asarray(weights, dtype=np.float32)
    wt = w.transpose(1, 2, 3, 0)  # [ci, ky, kx, co]
    w5 = np.zeros((128, 5, 128), np.float32)
    w5[0:64, 0:3, :] = wt[:, 0, :, :]
    w5[64:128, 0:3, :] = wt[:, 1, :, :]
    w5[0:64, 3, :] = wt[:, 2, 0, :]
    w5[64:128, 3, :] = wt[:, 2, 1, :]
    w5[64:128, 4, :] = wt[:, 2, 2, :]
    return w5.astype(np.float16)


def kernel(input_image: np.ndarray, weights: np.ndarray, _trace: bool = False):
    from concourse.bass_utils import run_bass_kernel_spmd

    nc = _get_nc()
    x16 = np.asarray(input_image).astype(np.float16)
    r = x16.reshape(B_FULL, C_IN, H * W)
    xd = np.zeros((B_FULL, 128, TLEN), np.float16)
    xd[:, 0:64] = r  # A: rows 0..111
    xd[:, 64:128, : TLEN - W] = r[:, :, W:]  # B: rows 1..111, zero pad
    w5 = _prep_weights(weights)
    in_maps = [
        {"x": xd[B_CORE * i : B_CORE * (i + 1)], "w": w5} for i in range(N_CORES)
    ]
    res = run_bass_kernel_spmd(
        nc, in_maps, core_ids=list(range(N_CORES)), trace=_trace
    )
    out = np.concatenate([res.results[i]["y"] for i in range(N_CORES)], axis=0)
    out = out.reshape(B_FULL, C_OUT, OH, OW).astype(np.float32)
    if _trace:
        return out, res
    return out
